# revision 6
# baseline (speedup 1.0000x reference)
"""Trainium2 Bass kernel for nn_NodeDecoder (sparse_attention).

Reference computation (B=256, V=16, N=1024, D=512):
    cat    = concat([g_node, Z_veh.mean(1), g_graph], -1)          # [B, 3D]
    ctx    = relu(cat @ W_ctx.T + b_ctx)                           # [B, D]
    Q      = ctx @ Wq.T                                            # [B, D]
    K      = Z_node @ Wk.T                                         # [B, N, D]
    logits = CLIP * tanh((Q . K) / sqrt(D)), masked to -inf        # [B, N]

Key algebraic transform: Q . (Z_node @ Wk.T) == (Q @ Wk) . Z_node, so the
B*N*D*D einsum collapses to a [B,D]@[D,D] matmul plus a B*N*D dot-product
sweep.  The kernel is then HBM-bandwidth-bound on streaming Z_node once.

Distribution: data-parallel over batch B across 8 NeuronCores (32 b/core),
small weights replicated.  All FLOPs (including the Z_veh mean, done as a
VectorE reduction) run on device; the host only slices/relayouts inputs and
reassembles the output (masked positions filled with -inf).

Per-core dataflow:
  - Z_veh mean: DVE tensor_reduce over V (1/V folded into W_ctx host-side)
  - build cat.T via PE transposes (identity matmuls)
  - chain ctx.T -> Q.T -> qtld.T = (Q @ Wk/sqrt(D)).T via PE matmuls with
    contraction on partitions (weights pre-transposed host-side: pure layout)
  - main loop over b: stream Z_node[b].T (host-relayouted [4,128,1024]) and
    accumulate logits[b, :] = qtld[:, b].T @ Zt in PSUM with the 128x1 q
    column as the stationary operand (float32r -> full-rate streaming),
    then tanh on ScalarE; rows bounce via a DRAM scratch and get a final
    xCLIP on VectorE.  M=1 keeps every engine SBUF access at partition 0.
"""

import numpy as np
from contextlib import ExitStack

B, V, N, D = 256, 16, 1024, 512
NCORES = 8
CLIP = 10.0
P = 128
DC = D // P          # 4 chunks of 128 along D
KC = (3 * D) // P    # 12 chunks along 3D
NH = N // 512        # moving-operand halves of the node dim

_CACHE = {}


def _build(BS, zbufs=8):
    """Build + compile the per-core Bass program for BS batches/core."""
    import concourse.bacc as bacc
    import concourse.tile as tile
    import concourse.mybir as mybir

    f32 = mybir.dt.float32
    f32r = mybir.dt.float32r
    Relu = mybir.ActivationFunctionType.Relu
    Tanh = mybir.ActivationFunctionType.Tanh
    Add = mybir.AluOpType.add
    AxX = mybir.AxisListType.X

    nc = bacc.Bacc("TRN2", target_bir_lowering=False, debug=False,
                   num_devices=NCORES)

    zt = nc.dram_tensor("zt", [BS, DC, P, N], f32r, kind="ExternalInput").ap()
    gn = nc.dram_tensor("gn", [BS, D], f32, kind="ExternalInput").ap()
    gg = nc.dram_tensor("gg", [BS, D], f32, kind="ExternalInput").ap()
    zv = nc.dram_tensor("zv", [BS, V, D], f32, kind="ExternalInput").ap()
    w1t = nc.dram_tensor("w1t", [3 * D, D], f32, kind="ExternalInput").ap()
    wqt = nc.dram_tensor("wqt", [D, D], f32, kind="ExternalInput").ap()
    wk = nc.dram_tensor("wk", [D, D], f32, kind="ExternalInput").ap()
    bc = nc.dram_tensor("bc", [D], f32, kind="ExternalInput").ap()
    eye = nc.dram_tensor("eye", [32, 32], f32, kind="ExternalInput").ap()
    out = nc.dram_tensor("out", [BS, N], f32, kind="ExternalOutput").ap()

    with tile.TileContext(nc) as tc, ExitStack() as ctx:
        singles = ctx.enter_context(tc.tile_pool(name="singles", bufs=1))
        qtld = singles.tile([P, DC, BS], f32r)

        pre_cm = tc.tile_pool(name="pre", bufs=1)
        pre = pre_cm.__enter__()
        pre_ps_cm = tc.tile_pool(name="pre_ps", bufs=2, space="PSUM")
        pre_ps = pre_ps_cm.__enter__()

        # ---- load replicated weights / per-core small inputs ----
        w1t_sb = pre.tile([P, KC, D], f32)
        nc.sync.dma_start(w1t_sb[:], w1t.rearrange("(kc p) j -> p kc j", p=P))
        wqt_sb = pre.tile([P, DC, D], f32)
        nc.sync.dma_start(wqt_sb[:], wqt.rearrange("(kc p) j -> p kc j", p=P))
        wk_sb = pre.tile([P, DC, D], f32)
        nc.sync.dma_start(wk_sb[:], wk.rearrange("(ec p) d -> p ec d", p=P))
        bc_sb = pre.tile([P, DC], f32)
        nc.sync.dma_start(bc_sb[:], bc.rearrange("(jc p) -> p jc", p=P))
        eye_sb = pre.tile([32, 32], f32)
        nc.sync.dma_start(eye_sb[:], eye[:])
        gn_sb = pre.tile([BS, D], f32)
        nc.sync.dma_start(gn_sb[:], gn[:])
        gg_sb = pre.tile([BS, D], f32)
        nc.sync.dma_start(gg_sb[:], gg[:])
        zv_sb = pre.tile([BS, V, D], f32)
        nc.sync.dma_start(zv_sb[:], zv[:])

        ident = eye_sb[:BS, :BS]

        # ---- Z_veh sum over V on VectorE (free-dim AP reorder) ----
        zvsum = pre.tile([BS, D], f32)
        nc.vector.tensor_reduce(zvsum[:], zv_sb[:].rearrange("p v d -> p d v"),
                                axis=AxX, op=Add)

        # ---- cat.T  [3D partition-chunks x BS] via PE transposes ----
        catT = pre.tile([P, KC, BS], f32)
        for dc in range(DC):
            ps = pre_ps.tile([P, BS], f32)
            nc.tensor.transpose(ps[:], gn_sb[:, dc * P:(dc + 1) * P], ident)
            nc.vector.tensor_copy(catT[:, dc, :], ps[:])
        for dc in range(DC):
            ps = pre_ps.tile([P, BS], f32)
            nc.tensor.transpose(ps[:], zvsum[:, dc * P:(dc + 1) * P], ident)
            nc.vector.tensor_copy(catT[:, DC + dc, :], ps[:])
        for dc in range(DC):
            ps = pre_ps.tile([P, BS], f32)
            nc.tensor.transpose(ps[:], gg_sb[:, dc * P:(dc + 1) * P], ident)
            nc.vector.tensor_copy(catT[:, 2 * DC + dc, :], ps[:])

        # ---- ctx.T = relu(W_ctx @ cat.T + b_ctx) ----
        ctxT = pre.tile([P, DC, BS], f32)
        for jc in range(DC):
            ps = pre_ps.tile([P, BS], f32)
            for kc in range(KC):
                nc.tensor.matmul(ps[:], w1t_sb[:, kc, jc * P:(jc + 1) * P],
                                 catT[:, kc, :],
                                 start=(kc == 0), stop=(kc == KC - 1))
            nc.scalar.activation(ctxT[:, jc, :], ps[:], Relu,
                                 bias=bc_sb[:, jc:jc + 1], scale=1.0)

        # ---- Q.T = Wq @ ctx.T ----
        qT = pre.tile([P, DC, BS], f32)
        for jc in range(DC):
            ps = pre_ps.tile([P, BS], f32)
            for kc in range(DC):
                nc.tensor.matmul(ps[:], wqt_sb[:, kc, jc * P:(jc + 1) * P],
                                 ctxT[:, kc, :],
                                 start=(kc == 0), stop=(kc == DC - 1))
            nc.vector.tensor_copy(qT[:, jc, :], ps[:])

        # ---- qtld.T = (Wk/sqrt(D)).T @ Q.T  (scale folded host-side) ----
        for dc in range(DC):
            ps = pre_ps.tile([P, BS], f32)
            for ec in range(DC):
                nc.tensor.matmul(ps[:], wk_sb[:, ec, dc * P:(dc + 1) * P],
                                 qT[:, ec, :],
                                 start=(ec == 0), stop=(ec == DC - 1))
            nc.vector.tensor_copy(qtld[:, dc, :], ps[:])

        pre_ps_cm.__exit__(None, None, None)
        pre_cm.__exit__(None, None, None)

        # ---- main loop: logits[b, :] = qtld[:, b] . Z_node[b].T ----
        zpool = ctx.enter_context(tc.tile_pool(name="z", bufs=zbufs))
        lps = ctx.enter_context(tc.tile_pool(name="lps", bufs=4, space="PSUM"))
        tpool = ctx.enter_context(tc.tile_pool(name="tanh", bufs=3))
        dpool = ctx.enter_context(tc.tile_pool(name="dram", bufs=1,
                                               space="DRAM"))
        lg = dpool.tile([BS, N], f32)
        for b in range(BS):
            ztile = zpool.tile([P, DC, N], f32r)
            nc.sync.dma_start(ztile[:], zt[b].rearrange("dc p n -> p dc n"))
            ps = lps.tile([1, N], f32)
            for dc in range(DC):
                for nh in range(NH):
                    nc.tensor.matmul(
                        ps[:, nh * 512:(nh + 1) * 512],
                        qtld[:, dc, b:b + 1],
                        ztile[:, dc, nh * 512:(nh + 1) * 512],
                        start=(dc == 0), stop=(dc == DC - 1))
            trow = tpool.tile([1, N], f32)
            nc.scalar.activation(trow[:], ps[:], Tanh, scale=1.0)
            nc.sync.dma_start(lg[b:b + 1, :], trow[:])
        ld = singles.tile([BS, N], f32)
        nc.sync.dma_start(ld[:], lg[:])
        out_sb = singles.tile([BS, N], f32)
        nc.vector.tensor_scalar_mul(out_sb[:], ld[:], CLIP)
        nc.sync.dma_start(out[:], out_sb[:])

    nc.compile()
    return nc


def _get_nc(BS):
    if BS not in _CACHE:
        _CACHE[BS] = _build(BS)
    return _CACHE[BS]


def _make_in_maps(g_node, Z_veh, g_graph, Z_node, W_ctx, b_ctx, Wq, Wk, BS):
    ncores = g_node.shape[0] // BS
    w1t = np.ascontiguousarray(W_ctx.T)          # [3D, D], k-major
    w1t[D:2 * D, :] *= np.float32(1.0 / V)       # fold the Z_veh mean's 1/V
    wqt = np.ascontiguousarray(Wq.T)             # [D, D], contraction-major
    wk = np.ascontiguousarray(Wk * np.float32(1.0 / np.sqrt(D)))
    bc = np.ascontiguousarray(b_ctx)
    eye = np.eye(32, dtype=np.float32)

    in_maps = []
    for c in range(ncores):
        s = slice(c * BS, (c + 1) * BS)
        zt = np.ascontiguousarray(
            Z_node[s].transpose(0, 2, 1)).reshape(BS, DC, P, N)
        in_maps.append({
            "zt": zt,
            "gn": np.ascontiguousarray(g_node[s]),
            "gg": np.ascontiguousarray(g_graph[s]),
            "zv": np.ascontiguousarray(Z_veh[s]),
            "w1t": w1t, "wqt": wqt, "wk": wk, "bc": bc, "eye": eye,
        })
    return in_maps


def kernel(g_node, Z_veh, g_graph, Z_node, mask, W_ctx, b_ctx, Wq, Wk):
    from concourse.bass_utils import run_bass_kernel_spmd

    g_node = np.asarray(g_node, np.float32)
    Z_veh = np.asarray(Z_veh, np.float32)
    g_graph = np.asarray(g_graph, np.float32)
    Z_node = np.asarray(Z_node, np.float32)
    mask = np.asarray(mask, bool)
    W_ctx = np.asarray(W_ctx, np.float32)
    b_ctx = np.asarray(b_ctx, np.float32)
    Wq = np.asarray(Wq, np.float32)
    Wk = np.asarray(Wk, np.float32)

    BS = B // NCORES
    nc = _get_nc(BS)
    in_maps = _make_in_maps(g_node, Z_veh, g_graph, Z_node,
                            W_ctx, b_ctx, Wq, Wk, BS)
    res = run_bass_kernel_spmd(nc, in_maps, core_ids=list(range(NCORES)))
    logits = np.concatenate([r["out"] for r in res.results], axis=0)
    return np.where(mask, logits, np.float32(-np.inf)).astype(np.float32)


# revision 9
# speedup vs baseline: 1.3288x; 1.3288x over previous
"""Trainium2 Bass kernel for nn_NodeDecoder (sparse_attention).

Reference computation (B=256, V=16, N=1024, D=512):
    cat    = concat([g_node, Z_veh.mean(1), g_graph], -1)          # [B, 3D]
    ctx    = relu(cat @ W_ctx.T + b_ctx)                           # [B, D]
    Q      = ctx @ Wq.T                                            # [B, D]
    K      = Z_node @ Wk.T                                         # [B, N, D]
    logits = CLIP * tanh((Q . K) / sqrt(D)), masked to -inf        # [B, N]

Key algebraic transform: Q . (Z_node @ Wk.T) == (Q @ Wk) . Z_node, so the
B*N*D*D einsum collapses to small matmuls plus a B*N*D dot-product sweep.
The kernel is then HBM-bandwidth-bound on streaming Z_node exactly once.
Wq.T @ Wk (weights only) is folded into a single matrix host-side, as are
the 1/V of the Z_veh mean and the 1/sqrt(D) logit scale.

Distribution: data-parallel over batch B across 8 NeuronCores (32 b/core),
small weights replicated.  All activation FLOPs run on device; the host
only slices/relayouts inputs, preprocesses weights, and reassembles the
output (masked positions filled with -inf).

Per-core dataflow:
  - Z_veh sum over V: VectorE tree adds (contiguous slices)
  - cat.T via 12 PE identity-transposes -> catT [128 x 12 x 32]
  - ctx  = relu(catT.T @ W1T + b): 12 accumulating PE matmuls with the
    [128,32] catT chunk stationary and the [128,512] weight chunk moving
    (float32r -> full-rate streaming), bias+relu on VectorE
  - qtld = ctx @ (Wq.T @ Wk)/sqrt(D): transpose ctx, 4 more matmuls,
    transpose back into qtldT [128 x 4 x 32]
  - main loop over b: stream Z_node[b].T (host-relayouted [4,128,1024]) on
    the Sync HWDGE ring (kept free of any other DMA so the Z stream never
    stalls behind compute waits) and accumulate logits[b, :] in PSUM with
    the 128x1 q column stationary; tanh on ScalarE; rows bounce via DRAM
    scratch (Scalar ring) and get a final xCLIP on VectorE.  M=1 keeps
    every engine SBUF access at partition 0/32/64/96.
"""

import numpy as np
from contextlib import ExitStack

B, V, N, D = 256, 16, 1024, 512
NCORES = 8
CLIP = 10.0
P = 128
DC = D // P          # 4 chunks of 128 along D
KC = (3 * D) // P    # 12 chunks along 3D
NH = N // 512        # moving-operand halves of the node dim

_CACHE = {}


def _build(BS, zbufs=6):
    """Build + compile the per-core Bass program for BS batches/core."""
    import concourse.bacc as bacc
    import concourse.tile as tile
    import concourse.mybir as mybir

    f32 = mybir.dt.float32
    f32r = mybir.dt.float32r
    Tanh = mybir.ActivationFunctionType.Tanh
    Add = mybir.AluOpType.add

    nc = bacc.Bacc("TRN2", target_bir_lowering=False, debug=False,
                   num_devices=NCORES)

    zt = nc.dram_tensor("zt", [BS, DC, P, N], f32r, kind="ExternalInput").ap()
    gn = nc.dram_tensor("gn", [BS, D], f32, kind="ExternalInput").ap()
    gg = nc.dram_tensor("gg", [BS, D], f32, kind="ExternalInput").ap()
    zv = nc.dram_tensor("zv", [BS, V, D], f32, kind="ExternalInput").ap()
    w1t = nc.dram_tensor("w1t", [3 * D, D], f32r, kind="ExternalInput").ap()
    wqk = nc.dram_tensor("wqk", [D, D], f32r, kind="ExternalInput").ap()
    brep = nc.dram_tensor("brep", [32, D], f32, kind="ExternalInput").ap()
    eye = nc.dram_tensor("eye", [32, 32], f32, kind="ExternalInput").ap()
    out = nc.dram_tensor("out", [BS, N], f32, kind="ExternalOutput").ap()

    with tile.TileContext(nc) as tc, ExitStack() as ctx:
        singles = ctx.enter_context(tc.tile_pool(name="singles", bufs=1))
        pre_ps_cm = tc.tile_pool(name="pre_ps", bufs=2, space="PSUM")
        pre_ps = pre_ps_cm.__enter__()

        # ---- load replicated weights / per-core small inputs ----
        # All preamble DMAs ride the Scalar HWDGE ring; the Sync ring is
        # reserved for the Z_node stream.
        w1t_sb = singles.tile([P, KC, D], f32r)
        nc.scalar.dma_start(w1t_sb[:],
                            w1t.rearrange("(kc p) j -> p kc j", p=P))
        wqk_sb = singles.tile([P, DC, D], f32r)
        nc.scalar.dma_start(wqk_sb[:],
                            wqk.rearrange("(kc p) j -> p kc j", p=P))
        brep_sb = singles.tile([32, D], f32)
        nc.scalar.dma_start(brep_sb[:], brep[:])
        eye_sb = singles.tile([32, 32], f32)
        nc.scalar.dma_start(eye_sb[:], eye[:])
        gn_sb = singles.tile([BS, D], f32)
        nc.scalar.dma_start(gn_sb[:], gn[:])
        gg_sb = singles.tile([BS, D], f32)
        nc.scalar.dma_start(gg_sb[:], gg[:])
        zv_sb = singles.tile([BS, V, D], f32)
        nc.scalar.dma_start(zv_sb[:], zv[:])

        ident = eye_sb[:BS, :BS]

        # ---- Z_veh sum over V: in-place VectorE tree adds ----
        h = V
        while h > 1:
            h //= 2
            nc.vector.tensor_tensor(zv_sb[:, 0:h, :], zv_sb[:, 0:h, :],
                                    zv_sb[:, h:2 * h, :], Add)
        zvsum = zv_sb[:, 0, :]

        # ---- cat.T  [3D partition-chunks x BS] via PE transposes ----
        catT = singles.tile([P, KC, BS], f32r)
        srcs = [gn_sb, zvsum, gg_sb]
        for g, src in enumerate(srcs):
            for dc in range(DC):
                tps = pre_ps.tile([P, BS], f32)
                nc.tensor.transpose(tps[:], src[:, dc * P:(dc + 1) * P], ident)
                nc.vector.tensor_copy(catT[:, g * DC + dc, :], tps[:])

        # ---- ctx = relu(cat @ W_ctx.T + b_ctx)   [32, 512] natural ----
        cps = pre_ps.tile([BS, D], f32)
        for kc in range(KC):
            nc.tensor.matmul(cps[:], catT[:, kc, :], w1t_sb[:, kc, :],
                             start=(kc == 0), stop=(kc == KC - 1))
        ctx_sb = singles.tile([BS, D], f32)
        nc.vector.tensor_tensor(ctx_sb[:], cps[:], brep_sb[:BS, :], Add)
        nc.vector.tensor_scalar_max(ctx_sb[:], ctx_sb[:], 0.0)

        # ---- ctx.T then qtld = ctx @ Wqk ----
        ctxT = singles.tile([P, DC, BS], f32r)
        for jc in range(DC):
            tps = pre_ps.tile([P, BS], f32)
            nc.tensor.transpose(tps[:], ctx_sb[:, jc * P:(jc + 1) * P], ident)
            nc.vector.tensor_copy(ctxT[:, jc, :], tps[:])
        qps = pre_ps.tile([BS, D], f32)
        for jc in range(DC):
            nc.tensor.matmul(qps[:], ctxT[:, jc, :], wqk_sb[:, jc, :],
                             start=(jc == 0), stop=(jc == DC - 1))
        qn_sb = singles.tile([BS, D], f32)
        nc.vector.tensor_copy(qn_sb[:], qps[:])

        # ---- qtld.T  [128 x 4 x 32] for the main-loop stationary ----
        qtldT = singles.tile([P, DC, BS], f32r)
        for dc in range(DC):
            tps = pre_ps.tile([P, BS], f32)
            nc.tensor.transpose(tps[:], qn_sb[:, dc * P:(dc + 1) * P], ident)
            nc.vector.tensor_copy(qtldT[:, dc, :], tps[:])

        pre_ps_cm.__exit__(None, None, None)

        # ---- main loop: logits[b, :] = qtldT[:, b] . Z_node[b].T ----
        zpool = ctx.enter_context(tc.tile_pool(name="z", bufs=zbufs))
        lps = ctx.enter_context(tc.tile_pool(name="lps", bufs=4, space="PSUM"))
        tpool = ctx.enter_context(tc.tile_pool(name="tanh", bufs=2))
        dpool = ctx.enter_context(tc.tile_pool(name="dram", bufs=1,
                                               space="DRAM"))
        lg = dpool.tile([BS, N], f32)
        for b in range(BS):
            ztile = zpool.tile([P, DC, N], f32r)
            nc.sync.dma_start(ztile[:], zt[b].rearrange("dc p n -> p dc n"))
            ps = lps.tile([1, N], f32)
            for dc in range(DC):
                for nh in range(NH):
                    nc.tensor.matmul(
                        ps[:, nh * 512:(nh + 1) * 512],
                        qtldT[:, dc, b:b + 1],
                        ztile[:, dc, nh * 512:(nh + 1) * 512],
                        start=(dc == 0), stop=(dc == DC - 1))
            trow = tpool.tile([1, N], f32)
            nc.scalar.activation(trow[:], ps[:], Tanh, scale=1.0)
            nc.scalar.dma_start(lg[b:b + 1, :], trow[:])
        ld = singles.tile([BS, N], f32)
        nc.scalar.dma_start(ld[:], lg[:])
        nc.vector.tensor_scalar_mul(ld[:], ld[:], CLIP)
        nc.scalar.dma_start(out[:], ld[:])

    nc.compile()
    return nc


def _get_nc(BS):
    if BS not in _CACHE:
        _CACHE[BS] = _build(BS)
    return _CACHE[BS]


def _make_in_maps(g_node, Z_veh, g_graph, Z_node, W_ctx, b_ctx, Wq, Wk, BS):
    ncores = g_node.shape[0] // BS
    w1t = np.ascontiguousarray(W_ctx.T)          # [3D, D], k-major
    w1t[D:2 * D, :] *= np.float32(1.0 / V)       # fold the Z_veh mean's 1/V
    # Weight-only fold: (ctx @ Wq.T) @ Wk == ctx @ (Wq.T @ Wk); also fold
    # the 1/sqrt(D) logit scale.  Computed in float64 for accuracy.
    wqk = ((Wq.T.astype(np.float64) @ Wk.astype(np.float64))
           / np.sqrt(D)).astype(np.float32)
    brep = np.broadcast_to(b_ctx, (32, D)).copy()
    eye = np.eye(32, dtype=np.float32)

    in_maps = []
    for c in range(ncores):
        s = slice(c * BS, (c + 1) * BS)
        ztc = np.ascontiguousarray(
            Z_node[s].transpose(0, 2, 1)).reshape(BS, DC, P, N)
        in_maps.append({
            "zt": ztc,
            "gn": np.ascontiguousarray(g_node[s]),
            "gg": np.ascontiguousarray(g_graph[s]),
            "zv": np.ascontiguousarray(Z_veh[s]),
            "w1t": w1t, "wqk": wqk, "brep": brep, "eye": eye,
        })
    return in_maps


def kernel(g_node, Z_veh, g_graph, Z_node, mask, W_ctx, b_ctx, Wq, Wk):
    from concourse.bass_utils import run_bass_kernel_spmd

    g_node = np.asarray(g_node, np.float32)
    Z_veh = np.asarray(Z_veh, np.float32)
    g_graph = np.asarray(g_graph, np.float32)
    Z_node = np.asarray(Z_node, np.float32)
    mask = np.asarray(mask, bool)
    W_ctx = np.asarray(W_ctx, np.float32)
    b_ctx = np.asarray(b_ctx, np.float32)
    Wq = np.asarray(Wq, np.float32)
    Wk = np.asarray(Wk, np.float32)

    BS = B // NCORES
    nc = _get_nc(BS)
    in_maps = _make_in_maps(g_node, Z_veh, g_graph, Z_node,
                            W_ctx, b_ctx, Wq, Wk, BS)
    res = run_bass_kernel_spmd(nc, in_maps, core_ids=list(range(NCORES)))
    logits = np.concatenate([r["out"] for r in res.results], axis=0)
    return np.where(mask, logits, np.float32(-np.inf)).astype(np.float32)


# revision 10
# speedup vs baseline: 1.3574x; 1.0215x over previous
"""Trainium2 Bass kernel for nn_NodeDecoder (sparse_attention).

Reference computation (B=256, V=16, N=1024, D=512):
    cat    = concat([g_node, Z_veh.mean(1), g_graph], -1)          # [B, 3D]
    ctx    = relu(cat @ W_ctx.T + b_ctx)                           # [B, D]
    Q      = ctx @ Wq.T                                            # [B, D]
    K      = Z_node @ Wk.T                                         # [B, N, D]
    logits = CLIP * tanh((Q . K) / sqrt(D)), masked to -inf        # [B, N]

Key algebraic transform: Q . (Z_node @ Wk.T) == (Q @ Wk) . Z_node, so the
B*N*D*D einsum collapses to small matmuls plus a B*N*D dot-product sweep.
The kernel is then HBM-bandwidth-bound on streaming Z_node exactly once.
Wq.T @ Wk (weights only) is folded into a single matrix host-side, as are
the 1/V of the Z_veh mean and the 1/sqrt(D) logit scale.

Distribution: data-parallel over batch B across 8 NeuronCores (32 b/core),
small weights replicated.  All activation FLOPs run on device; the host
only slices/relayouts inputs, preprocesses weights, and reassembles the
output (masked positions filled with -inf).

Per-core dataflow:
  - Z_veh sum over V: VectorE tree adds (contiguous slices)
  - cat.T via 12 PE identity-transposes -> catT [128 x 12 x 32]
  - ctx  = relu(catT.T @ W1T + b): 12 accumulating PE matmuls with the
    [128,32] catT chunk stationary and the [128,512] weight chunk moving
    (float32r -> full-rate streaming), bias+relu on VectorE
  - qtld = ctx @ (Wq.T @ Wk)/sqrt(D): transpose ctx, 4 more matmuls,
    transpose back into qtldT [128 x 4 x 32]
  - main loop over b: stream Z_node[b].T (host-relayouted [4,128,1024]) on
    the Sync HWDGE ring (kept free of any other DMA so the Z stream never
    stalls behind compute waits) and accumulate logits[b, :] in PSUM with
    the 128x1 q column stationary; tanh on ScalarE; rows bounce via DRAM
    scratch (Scalar ring) and get a final xCLIP on VectorE.  M=1 keeps
    every engine SBUF access at partition 0/32/64/96.
"""

import numpy as np
from contextlib import ExitStack

B, V, N, D = 256, 16, 1024, 512
NCORES = 8
CLIP = 10.0
P = 128
DC = D // P          # 4 chunks of 128 along D
KC = (3 * D) // P    # 12 chunks along 3D
NH = N // 512        # moving-operand halves of the node dim

_CACHE = {}


def _build(BS, zbufs=6):
    """Build + compile the per-core Bass program for BS batches/core."""
    import concourse.bacc as bacc
    import concourse.tile as tile
    import concourse.mybir as mybir

    f32 = mybir.dt.float32
    f32r = mybir.dt.float32r
    Tanh = mybir.ActivationFunctionType.Tanh
    Add = mybir.AluOpType.add

    nc = bacc.Bacc("TRN2", target_bir_lowering=False, debug=False,
                   num_devices=NCORES)

    zt = nc.dram_tensor("zt", [BS, DC, P, N], f32r, kind="ExternalInput").ap()
    gn = nc.dram_tensor("gn", [BS, D], f32, kind="ExternalInput").ap()
    gg = nc.dram_tensor("gg", [BS, D], f32, kind="ExternalInput").ap()
    zv = nc.dram_tensor("zv", [BS, V, D], f32, kind="ExternalInput").ap()
    w1t = nc.dram_tensor("w1t", [3 * D, D], f32r, kind="ExternalInput").ap()
    wqk = nc.dram_tensor("wqk", [D, D], f32r, kind="ExternalInput").ap()
    brep = nc.dram_tensor("brep", [32, D], f32, kind="ExternalInput").ap()
    eye = nc.dram_tensor("eye", [32, 32], f32, kind="ExternalInput").ap()
    out = nc.dram_tensor("out", [BS, N], f32, kind="ExternalOutput").ap()

    with tile.TileContext(nc) as tc, ExitStack() as ctx:
        singles = ctx.enter_context(tc.tile_pool(name="singles", bufs=1))
        pre_cm = tc.tile_pool(name="pre", bufs=1)
        pre = pre_cm.__enter__()
        pre_ps_cm = tc.tile_pool(name="pre_ps", bufs=2, space="PSUM")
        pre_ps = pre_ps_cm.__enter__()

        # ---- load replicated weights / per-core small inputs ----
        # All preamble DMAs ride the Scalar HWDGE ring; the Sync ring is
        # reserved for the Z_node stream.
        zv_sb = pre.tile([BS, V, D], f32)
        nc.scalar.dma_start(zv_sb[:], zv[:])
        gn_sb = pre.tile([BS, D], f32)
        nc.scalar.dma_start(gn_sb[:], gn[:])
        gg_sb = pre.tile([BS, D], f32)
        nc.scalar.dma_start(gg_sb[:], gg[:])
        eye_sb = pre.tile([32, 32], f32)
        nc.scalar.dma_start(eye_sb[:], eye[:])
        brep_sb = pre.tile([32, D], f32)
        nc.scalar.dma_start(brep_sb[:], brep[:])
        w1t_sb = pre.tile([P, KC, D], f32r)
        nc.scalar.dma_start(w1t_sb[:],
                            w1t.rearrange("(kc p) j -> p kc j", p=P))
        wqk_sb = pre.tile([P, DC, D], f32r)
        nc.scalar.dma_start(wqk_sb[:],
                            wqk.rearrange("(kc p) j -> p kc j", p=P))

        ident = eye_sb[:BS, :BS]

        # ---- Z_veh sum over V: in-place VectorE tree adds ----
        h = V
        while h > 1:
            h //= 2
            nc.vector.tensor_tensor(zv_sb[:, 0:h, :], zv_sb[:, 0:h, :],
                                    zv_sb[:, h:2 * h, :], Add)
        zvsum = zv_sb[:, 0, :]

        # ---- cat.T  [3D partition-chunks x BS] via PE transposes ----
        catT = singles.tile([P, KC, BS], f32r)
        srcs = [gn_sb, zvsum, gg_sb]
        for g, src in enumerate(srcs):
            for dc in range(DC):
                tps = pre_ps.tile([P, BS], f32)
                nc.tensor.transpose(tps[:], src[:, dc * P:(dc + 1) * P], ident)
                nc.vector.tensor_copy(catT[:, g * DC + dc, :], tps[:])

        # ---- ctx = relu(cat @ W_ctx.T + b_ctx)   [32, 512] natural ----
        cps = pre_ps.tile([BS, D], f32)
        for kc in range(KC):
            nc.tensor.matmul(cps[:], catT[:, kc, :], w1t_sb[:, kc, :],
                             start=(kc == 0), stop=(kc == KC - 1))
        ctx_sb = pre.tile([BS, D], f32)
        nc.vector.tensor_tensor(ctx_sb[:], cps[:], brep_sb[:BS, :], Add)
        nc.vector.tensor_scalar_max(ctx_sb[:], ctx_sb[:], 0.0)

        # ---- ctx.T then qtld = ctx @ Wqk ----
        ctxT = pre.tile([P, DC, BS], f32r)
        for jc in range(DC):
            tps = pre_ps.tile([P, BS], f32)
            nc.tensor.transpose(tps[:], ctx_sb[:, jc * P:(jc + 1) * P], ident)
            nc.vector.tensor_copy(ctxT[:, jc, :], tps[:])
        qps = pre_ps.tile([BS, D], f32)
        for jc in range(DC):
            nc.tensor.matmul(qps[:], ctxT[:, jc, :], wqk_sb[:, jc, :],
                             start=(jc == 0), stop=(jc == DC - 1))
        qn_sb = pre.tile([BS, D], f32)
        nc.vector.tensor_copy(qn_sb[:], qps[:])

        # ---- qtld.T  [128 x 4 x 32] for the main-loop stationary ----
        qtldT = singles.tile([P, DC, BS], f32r)
        for dc in range(DC):
            tps = pre_ps.tile([P, BS], f32)
            nc.tensor.transpose(tps[:], qn_sb[:, dc * P:(dc + 1) * P], ident)
            nc.vector.tensor_copy(qtldT[:, dc, :], tps[:])

        pre_ps_cm.__exit__(None, None, None)
        pre_cm.__exit__(None, None, None)

        # ---- main loop: logits[b, :] = qtldT[:, b] . Z_node[b].T ----
        # Hybrid double buffer pool: zpool sits in virgin SBUF (prefetch can
        # start at t=0), zpool2 reuses the preamble region (its first tiles
        # wait for the preamble tiles' last readers, which is fine: they are
        # only needed once the pipeline is deep).
        zpool = ctx.enter_context(tc.tile_pool(name="z", bufs=zbufs))
        zpool2 = ctx.enter_context(tc.tile_pool(name="z2", bufs=4))
        lps = ctx.enter_context(tc.tile_pool(name="lps", bufs=4, space="PSUM"))
        tpool = ctx.enter_context(tc.tile_pool(name="tanh", bufs=2))
        dpool = ctx.enter_context(tc.tile_pool(name="dram", bufs=1,
                                               space="DRAM"))
        lg = dpool.tile([BS, N], f32)
        for b in range(BS):
            zp = zpool if (b % 10) < zbufs else zpool2
            ztile = zp.tile([P, DC, N], f32r)
            nc.sync.dma_start(ztile[:], zt[b].rearrange("dc p n -> p dc n"))
            ps = lps.tile([1, N], f32)
            for dc in range(DC):
                for nh in range(NH):
                    nc.tensor.matmul(
                        ps[:, nh * 512:(nh + 1) * 512],
                        qtldT[:, dc, b:b + 1],
                        ztile[:, dc, nh * 512:(nh + 1) * 512],
                        start=(dc == 0), stop=(dc == DC - 1))
            trow = tpool.tile([1, N], f32)
            nc.scalar.activation(trow[:], ps[:], Tanh, scale=1.0)
            nc.scalar.dma_start(lg[b:b + 1, :], trow[:])
        ld = singles.tile([BS, N], f32)
        nc.scalar.dma_start(ld[:], lg[:])
        nc.vector.tensor_scalar_mul(ld[:], ld[:], CLIP)
        nc.scalar.dma_start(out[:], ld[:])

    nc.compile()
    return nc


def _get_nc(BS):
    if BS not in _CACHE:
        _CACHE[BS] = _build(BS)
    return _CACHE[BS]


def _make_in_maps(g_node, Z_veh, g_graph, Z_node, W_ctx, b_ctx, Wq, Wk, BS):
    ncores = g_node.shape[0] // BS
    w1t = np.ascontiguousarray(W_ctx.T)          # [3D, D], k-major
    w1t[D:2 * D, :] *= np.float32(1.0 / V)       # fold the Z_veh mean's 1/V
    # Weight-only fold: (ctx @ Wq.T) @ Wk == ctx @ (Wq.T @ Wk); also fold
    # the 1/sqrt(D) logit scale.  Computed in float64 for accuracy.
    wqk = ((Wq.T.astype(np.float64) @ Wk.astype(np.float64))
           / np.sqrt(D)).astype(np.float32)
    brep = np.broadcast_to(b_ctx, (32, D)).copy()
    eye = np.eye(32, dtype=np.float32)

    in_maps = []
    for c in range(ncores):
        s = slice(c * BS, (c + 1) * BS)
        ztc = np.ascontiguousarray(
            Z_node[s].transpose(0, 2, 1)).reshape(BS, DC, P, N)
        in_maps.append({
            "zt": ztc,
            "gn": np.ascontiguousarray(g_node[s]),
            "gg": np.ascontiguousarray(g_graph[s]),
            "zv": np.ascontiguousarray(Z_veh[s]),
            "w1t": w1t, "wqk": wqk, "brep": brep, "eye": eye,
        })
    return in_maps


def kernel(g_node, Z_veh, g_graph, Z_node, mask, W_ctx, b_ctx, Wq, Wk):
    from concourse.bass_utils import run_bass_kernel_spmd

    g_node = np.asarray(g_node, np.float32)
    Z_veh = np.asarray(Z_veh, np.float32)
    g_graph = np.asarray(g_graph, np.float32)
    Z_node = np.asarray(Z_node, np.float32)
    mask = np.asarray(mask, bool)
    W_ctx = np.asarray(W_ctx, np.float32)
    b_ctx = np.asarray(b_ctx, np.float32)
    Wq = np.asarray(Wq, np.float32)
    Wk = np.asarray(Wk, np.float32)

    BS = B // NCORES
    nc = _get_nc(BS)
    in_maps = _make_in_maps(g_node, Z_veh, g_graph, Z_node,
                            W_ctx, b_ctx, Wq, Wk, BS)
    res = run_bass_kernel_spmd(nc, in_maps, core_ids=list(range(NCORES)))
    logits = np.concatenate([r["out"] for r in res.results], axis=0)
    return np.where(mask, logits, np.float32(-np.inf)).astype(np.float32)


# revision 11
# speedup vs baseline: 1.5267x; 1.1248x over previous
"""Trainium2 Bass kernel for nn_NodeDecoder (sparse_attention).

Reference computation (B=256, V=16, N=1024, D=512):
    cat    = concat([g_node, Z_veh.mean(1), g_graph], -1)          # [B, 3D]
    ctx    = relu(cat @ W_ctx.T + b_ctx)                           # [B, D]
    Q      = ctx @ Wq.T                                            # [B, D]
    K      = Z_node @ Wk.T                                         # [B, N, D]
    logits = CLIP * tanh((Q . K) / sqrt(D)), masked to -inf        # [B, N]

Key algebraic transform: Q . (Z_node @ Wk.T) == (Q @ Wk) . Z_node, so the
B*N*D*D einsum collapses to small matmuls plus a B*N*D dot-product sweep.
The kernel is then HBM-bandwidth-bound on streaming Z_node exactly once.
Wq.T @ Wk (weights only) is folded into a single matrix host-side, as are
the 1/V of the Z_veh mean and the 1/sqrt(D) logit scale.

Distribution: data-parallel over batch B across 8 NeuronCores (32 b/core),
small weights replicated.  All activation FLOPs run on device; the host
only slices/relayouts inputs, preprocesses weights, and reassembles the
output (masked positions filled with -inf).

Per-core dataflow:
  - Z_veh sum over V: VectorE tree adds (contiguous slices)
  - cat.T via 12 PE identity-transposes -> catT [128 x 12 x 32]
  - ctx  = relu(catT.T @ W1T + b): 12 accumulating PE matmuls with the
    [128,32] catT chunk stationary and the [128,512] weight chunk moving
    (float32r -> full-rate streaming), bias+relu on VectorE
  - qtld = ctx @ (Wq.T @ Wk)/sqrt(D): transpose ctx, 4 more matmuls,
    transpose back into qtldT [128 x 4 x 32]
  - main loop over b: stream Z_node[b].T (host-relayouted [4,128,1024]) on
    the Sync HWDGE ring (kept free of any other DMA so the Z stream never
    stalls behind compute waits) and accumulate logits[b, :] in PSUM with
    the 128x1 q column stationary; tanh on ScalarE; rows bounce via DRAM
    scratch (Scalar ring) and get a final xCLIP on VectorE.  M=1 keeps
    every engine SBUF access at partition 0/32/64/96.
"""

import numpy as np
from contextlib import ExitStack

B, V, N, D = 256, 16, 1024, 512
NCORES = 8
CLIP = 10.0
P = 128
DC = D // P          # 4 chunks of 128 along D
KC = (3 * D) // P    # 12 chunks along 3D
NH = N // 512        # moving-operand halves of the node dim

_CACHE = {}


def _build(BS, zbufs=6):
    """Build + compile the per-core Bass program for BS batches/core."""
    import concourse.bacc as bacc
    import concourse.tile as tile
    import concourse.mybir as mybir

    f32 = mybir.dt.float32
    f32r = mybir.dt.float32r
    Tanh = mybir.ActivationFunctionType.Tanh
    Add = mybir.AluOpType.add

    nc = bacc.Bacc("TRN2", target_bir_lowering=False, debug=False,
                   num_devices=NCORES)

    zt = nc.dram_tensor("zt", [BS, DC, P, N], f32r, kind="ExternalInput").ap()
    gn = nc.dram_tensor("gn", [BS, D], f32, kind="ExternalInput").ap()
    gg = nc.dram_tensor("gg", [BS, D], f32, kind="ExternalInput").ap()
    zv = nc.dram_tensor("zv", [BS, V, D], f32, kind="ExternalInput").ap()
    w1t = nc.dram_tensor("w1t", [3 * D, D], f32r, kind="ExternalInput").ap()
    wqk = nc.dram_tensor("wqk", [D, D], f32r, kind="ExternalInput").ap()
    brep = nc.dram_tensor("brep", [32, D], f32, kind="ExternalInput").ap()
    eye = nc.dram_tensor("eye", [32, 32], f32, kind="ExternalInput").ap()
    out = nc.dram_tensor("out", [BS, N], f32, kind="ExternalOutput").ap()

    with tile.TileContext(nc) as tc, ExitStack() as ctx:
        singles = ctx.enter_context(tc.tile_pool(name="singles", bufs=1))
        pre_cm = tc.tile_pool(name="pre", bufs=1)
        pre = pre_cm.__enter__()
        pre_ps_cm = tc.tile_pool(name="pre_ps", bufs=2, space="PSUM")
        pre_ps = pre_ps_cm.__enter__()

        # ---- load replicated weights / per-core small inputs ----
        # All preamble DMAs sit at the HEAD of the Sync HWDGE ring, before
        # the Z_node stream: strict FIFO order means the (critical-path)
        # weights are delivered at full bandwidth instead of being starved
        # by a competing saturating queue.
        eye_sb = pre.tile([32, 32], f32)
        nc.sync.dma_start(eye_sb[:], eye[:])
        zv_sb = pre.tile([BS, V, D], f32)
        nc.sync.dma_start(zv_sb[:], zv[:])
        gn_sb = pre.tile([BS, D], f32)
        nc.sync.dma_start(gn_sb[:], gn[:])
        gg_sb = pre.tile([BS, D], f32)
        nc.sync.dma_start(gg_sb[:], gg[:])
        brep_sb = pre.tile([32, D], f32)
        nc.sync.dma_start(brep_sb[:], brep[:])
        w1t_sb = pre.tile([P, KC, D], f32r)
        nc.sync.dma_start(w1t_sb[:],
                          w1t.rearrange("(kc p) j -> p kc j", p=P))
        wqk_sb = pre.tile([P, DC, D], f32r)
        nc.sync.dma_start(wqk_sb[:],
                          wqk.rearrange("(kc p) j -> p kc j", p=P))

        ident = eye_sb[:BS, :BS]

        # ---- Z_veh sum over V: in-place VectorE tree adds ----
        h = V
        while h > 1:
            h //= 2
            nc.vector.tensor_tensor(zv_sb[:, 0:h, :], zv_sb[:, 0:h, :],
                                    zv_sb[:, h:2 * h, :], Add)
        zvsum = zv_sb[:, 0, :]

        # ---- cat.T  [3D partition-chunks x BS] via PE transposes ----
        catT = singles.tile([P, KC, BS], f32r)
        srcs = [gn_sb, zvsum, gg_sb]
        for g, src in enumerate(srcs):
            for dc in range(DC):
                tps = pre_ps.tile([P, BS], f32)
                nc.tensor.transpose(tps[:], src[:, dc * P:(dc + 1) * P], ident)
                nc.vector.tensor_copy(catT[:, g * DC + dc, :], tps[:])

        # ---- ctx = relu(cat @ W_ctx.T + b_ctx)   [32, 512] natural ----
        cps = pre_ps.tile([BS, D], f32)
        for kc in range(KC):
            nc.tensor.matmul(cps[:], catT[:, kc, :], w1t_sb[:, kc, :],
                             start=(kc == 0), stop=(kc == KC - 1))
        ctx_sb = pre.tile([BS, D], f32)
        nc.vector.tensor_tensor(ctx_sb[:], cps[:], brep_sb[:BS, :], Add)
        nc.vector.tensor_scalar_max(ctx_sb[:], ctx_sb[:], 0.0)

        # ---- ctx.T then qtld = ctx @ Wqk ----
        ctxT = pre.tile([P, DC, BS], f32r)
        for jc in range(DC):
            tps = pre_ps.tile([P, BS], f32)
            nc.tensor.transpose(tps[:], ctx_sb[:, jc * P:(jc + 1) * P], ident)
            nc.vector.tensor_copy(ctxT[:, jc, :], tps[:])
        qps = pre_ps.tile([BS, D], f32)
        for jc in range(DC):
            nc.tensor.matmul(qps[:], ctxT[:, jc, :], wqk_sb[:, jc, :],
                             start=(jc == 0), stop=(jc == DC - 1))
        qn_sb = pre.tile([BS, D], f32)
        nc.vector.tensor_copy(qn_sb[:], qps[:])

        # ---- qtld.T  [128 x 4 x 32] for the main-loop stationary ----
        qtldT = singles.tile([P, DC, BS], f32r)
        for dc in range(DC):
            tps = pre_ps.tile([P, BS], f32)
            nc.tensor.transpose(tps[:], qn_sb[:, dc * P:(dc + 1) * P], ident)
            nc.vector.tensor_copy(qtldT[:, dc, :], tps[:])

        pre_ps_cm.__exit__(None, None, None)
        pre_cm.__exit__(None, None, None)

        # ---- main loop: logits[b, :] = qtldT[:, b] . Z_node[b].T ----
        zpool = ctx.enter_context(tc.tile_pool(name="z", bufs=zbufs))
        lps = ctx.enter_context(tc.tile_pool(name="lps", bufs=4, space="PSUM"))
        tpool = ctx.enter_context(tc.tile_pool(name="tanh", bufs=2))
        dpool = ctx.enter_context(tc.tile_pool(name="dram", bufs=1,
                                               space="DRAM"))
        lg = dpool.tile([BS, N], f32)
        for b in range(BS):
            ztile = zpool.tile([P, DC, N], f32r)
            nc.sync.dma_start(ztile[:], zt[b].rearrange("dc p n -> p dc n"))
            ps = lps.tile([1, N], f32)
            for dc in range(DC):
                for nh in range(NH):
                    nc.tensor.matmul(
                        ps[:, nh * 512:(nh + 1) * 512],
                        qtldT[:, dc, b:b + 1],
                        ztile[:, dc, nh * 512:(nh + 1) * 512],
                        start=(dc == 0), stop=(dc == DC - 1))
            trow = tpool.tile([1, N], f32)
            nc.scalar.activation(trow[:], ps[:], Tanh, scale=1.0)
            nc.scalar.dma_start(lg[b:b + 1, :], trow[:])
        ld = singles.tile([BS, N], f32)
        nc.scalar.dma_start(ld[:], lg[:])
        nc.vector.tensor_scalar_mul(ld[:], ld[:], CLIP)
        nc.scalar.dma_start(out[:], ld[:])

    nc.compile()
    return nc


def _get_nc(BS):
    if BS not in _CACHE:
        _CACHE[BS] = _build(BS)
    return _CACHE[BS]


def _make_in_maps(g_node, Z_veh, g_graph, Z_node, W_ctx, b_ctx, Wq, Wk, BS):
    ncores = g_node.shape[0] // BS
    w1t = np.ascontiguousarray(W_ctx.T)          # [3D, D], k-major
    w1t[D:2 * D, :] *= np.float32(1.0 / V)       # fold the Z_veh mean's 1/V
    # Weight-only fold: (ctx @ Wq.T) @ Wk == ctx @ (Wq.T @ Wk); also fold
    # the 1/sqrt(D) logit scale.  Computed in float64 for accuracy.
    wqk = ((Wq.T.astype(np.float64) @ Wk.astype(np.float64))
           / np.sqrt(D)).astype(np.float32)
    brep = np.broadcast_to(b_ctx, (32, D)).copy()
    eye = np.eye(32, dtype=np.float32)

    in_maps = []
    for c in range(ncores):
        s = slice(c * BS, (c + 1) * BS)
        ztc = np.ascontiguousarray(
            Z_node[s].transpose(0, 2, 1)).reshape(BS, DC, P, N)
        in_maps.append({
            "zt": ztc,
            "gn": np.ascontiguousarray(g_node[s]),
            "gg": np.ascontiguousarray(g_graph[s]),
            "zv": np.ascontiguousarray(Z_veh[s]),
            "w1t": w1t, "wqk": wqk, "brep": brep, "eye": eye,
        })
    return in_maps


def kernel(g_node, Z_veh, g_graph, Z_node, mask, W_ctx, b_ctx, Wq, Wk):
    from concourse.bass_utils import run_bass_kernel_spmd

    g_node = np.asarray(g_node, np.float32)
    Z_veh = np.asarray(Z_veh, np.float32)
    g_graph = np.asarray(g_graph, np.float32)
    Z_node = np.asarray(Z_node, np.float32)
    mask = np.asarray(mask, bool)
    W_ctx = np.asarray(W_ctx, np.float32)
    b_ctx = np.asarray(b_ctx, np.float32)
    Wq = np.asarray(Wq, np.float32)
    Wk = np.asarray(Wk, np.float32)

    BS = B // NCORES
    nc = _get_nc(BS)
    in_maps = _make_in_maps(g_node, Z_veh, g_graph, Z_node,
                            W_ctx, b_ctx, Wq, Wk, BS)
    res = run_bass_kernel_spmd(nc, in_maps, core_ids=list(range(NCORES)))
    logits = np.concatenate([r["out"] for r in res.results], axis=0)
    return np.where(mask, logits, np.float32(-np.inf)).astype(np.float32)


# revision 13
# speedup vs baseline: 1.7066x; 1.1178x over previous
"""Trainium2 Bass kernel for nn_NodeDecoder (sparse_attention).

Reference computation (B=256, V=16, N=1024, D=512):
    cat    = concat([g_node, Z_veh.mean(1), g_graph], -1)          # [B, 3D]
    ctx    = relu(cat @ W_ctx.T + b_ctx)                           # [B, D]
    Q      = ctx @ Wq.T                                            # [B, D]
    K      = Z_node @ Wk.T                                         # [B, N, D]
    logits = CLIP * tanh((Q . K) / sqrt(D)), masked to -inf        # [B, N]

Key algebraic transform: Q . (Z_node @ Wk.T) == (Q @ Wk) . Z_node, so the
B*N*D*D einsum collapses to small matmuls plus a B*N*D dot-product sweep.
The kernel is then HBM-bandwidth-bound on streaming Z_node exactly once.
Wq.T @ Wk (weights only) is folded into a single matrix host-side, as are
the 1/V of the Z_veh mean and the 1/sqrt(D) logit scale.

Distribution: data-parallel over batch B across 8 NeuronCores (32 b/core),
small weights replicated.  All activation FLOPs run on device; the host
only slices/relayouts inputs, preprocesses weights, and reassembles the
output (masked positions filled with -inf).

Per-core dataflow:
  - Z_veh sum over V: VectorE tree adds (contiguous slices)
  - cat.T via 12 PE identity-transposes -> catT [128 x 12 x 32]
  - ctx  = relu(catT.T @ W1T + b): 12 accumulating PE matmuls with the
    [128,32] catT chunk stationary and the [128,512] weight chunk moving
    (float32r -> full-rate streaming), bias+relu on VectorE
  - qtld = ctx @ (Wq.T @ Wk)/sqrt(D): transpose ctx, 4 more matmuls,
    transpose back into qtldT [128 x 4 x 32]
  - main loop over b: stream Z_node[b].T (host-relayouted [4,128,1024]) on
    the Sync HWDGE ring (kept free of any other DMA so the Z stream never
    stalls behind compute waits) and accumulate logits[b, :] in PSUM with
    the 128x1 q column stationary; tanh on ScalarE; rows bounce via DRAM
    scratch (Scalar ring) and get a final xCLIP on VectorE.  M=1 keeps
    every engine SBUF access at partition 0/32/64/96.
"""

import numpy as np
from contextlib import ExitStack

B, V, N, D = 256, 16, 1024, 512
NCORES = 8
CLIP = 10.0
P = 128
DC = D // P          # 4 chunks of 128 along D
KC = (3 * D) // P    # 12 chunks along 3D
NH = N // 512        # moving-operand halves of the node dim

_CACHE = {}


def _build(BS, zbufs=6):
    """Build + compile the per-core Bass program for BS batches/core."""
    import concourse.bacc as bacc
    import concourse.tile as tile
    import concourse.mybir as mybir

    f32 = mybir.dt.float32
    f32r = mybir.dt.float32r
    Tanh = mybir.ActivationFunctionType.Tanh
    Add = mybir.AluOpType.add

    nc = bacc.Bacc("TRN2", target_bir_lowering=False, debug=False,
                   num_devices=NCORES)

    zt = nc.dram_tensor("zt", [BS, DC, P, N], f32r, kind="ExternalInput").ap()
    gn = nc.dram_tensor("gn", [BS, D], f32, kind="ExternalInput").ap()
    gg = nc.dram_tensor("gg", [BS, D], f32, kind="ExternalInput").ap()
    zv = nc.dram_tensor("zv", [BS, V, D], f32, kind="ExternalInput").ap()
    w1t = nc.dram_tensor("w1t", [3 * D, D], f32r, kind="ExternalInput").ap()
    wqk = nc.dram_tensor("wqk", [D, D], f32r, kind="ExternalInput").ap()
    brep = nc.dram_tensor("brep", [32, D], f32, kind="ExternalInput").ap()
    eye = nc.dram_tensor("eye", [32, 32], f32, kind="ExternalInput").ap()
    out = nc.dram_tensor("out", [BS, N], f32, kind="ExternalOutput").ap()

    with tile.TileContext(nc) as tc, ExitStack() as ctx:
        singles = ctx.enter_context(tc.tile_pool(name="singles", bufs=1))
        pre = ctx.enter_context(tc.tile_pool(name="pre", bufs=1))
        pre_ps_cm = tc.tile_pool(name="pre_ps", bufs=2, space="PSUM")
        pre_ps = pre_ps_cm.__enter__()

        # ---- load replicated weights / per-core small inputs ----
        # All preamble DMAs sit at the HEAD of the Sync HWDGE ring, before
        # the Z_node stream: strict FIFO order means the (critical-path)
        # weights are delivered at full bandwidth instead of being starved
        # by a competing saturating queue.
        eye_sb = pre.tile([32, 32], f32)
        nc.sync.dma_start(eye_sb[:], eye[:])
        zv_sb = pre.tile([BS, V, D], f32)
        nc.sync.dma_start(zv_sb[:], zv[:])
        gn_sb = pre.tile([BS, D], f32)
        nc.sync.dma_start(gn_sb[:], gn[:])
        gg_sb = pre.tile([BS, D], f32)
        nc.sync.dma_start(gg_sb[:], gg[:])
        brep_sb = pre.tile([32, D], f32)
        nc.sync.dma_start(brep_sb[:], brep[:])
        w1t_sb = pre.tile([P, KC, D], f32r)
        nc.sync.dma_start(w1t_sb[:],
                          w1t.rearrange("(kc p) j -> p kc j", p=P))
        wqk_sb = pre.tile([P, DC, D], f32r)
        nc.sync.dma_start(wqk_sb[:],
                          wqk.rearrange("(kc p) j -> p kc j", p=P))

        ident = eye_sb[:BS, :BS]

        # ---- Z_veh sum over V: in-place VectorE tree adds ----
        h = V
        while h > 1:
            h //= 2
            nc.vector.tensor_tensor(zv_sb[:, 0:h, :], zv_sb[:, 0:h, :],
                                    zv_sb[:, h:2 * h, :], Add)
        zvsum = zv_sb[:, 0, :]

        # ---- cat.T  [3D partition-chunks x BS] via PE transposes ----
        catT = singles.tile([P, KC, BS], f32r)
        srcs = [gn_sb, zvsum, gg_sb]
        for g, src in enumerate(srcs):
            for dc in range(DC):
                tps = pre_ps.tile([P, BS], f32)
                nc.tensor.transpose(tps[:], src[:, dc * P:(dc + 1) * P], ident)
                nc.vector.tensor_copy(catT[:, g * DC + dc, :], tps[:])

        # ---- ctx = relu(cat @ W_ctx.T + b_ctx)   [32, 512] natural ----
        cps = pre_ps.tile([BS, D], f32)
        for kc in range(KC):
            nc.tensor.matmul(cps[:], catT[:, kc, :], w1t_sb[:, kc, :],
                             start=(kc == 0), stop=(kc == KC - 1))
        ctx_sb = pre.tile([BS, D], f32)
        nc.vector.tensor_tensor(ctx_sb[:], cps[:], brep_sb[:BS, :], Add)
        nc.vector.tensor_scalar_max(ctx_sb[:], ctx_sb[:], 0.0)

        # ---- ctx.T then qtld = ctx @ Wqk ----
        ctxT = pre.tile([P, DC, BS], f32r)
        for jc in range(DC):
            tps = pre_ps.tile([P, BS], f32)
            nc.tensor.transpose(tps[:], ctx_sb[:, jc * P:(jc + 1) * P], ident)
            nc.vector.tensor_copy(ctxT[:, jc, :], tps[:])
        qps = pre_ps.tile([BS, D], f32)
        for jc in range(DC):
            nc.tensor.matmul(qps[:], ctxT[:, jc, :], wqk_sb[:, jc, :],
                             start=(jc == 0), stop=(jc == DC - 1))
        qn_sb = pre.tile([BS, D], f32)
        nc.vector.tensor_copy(qn_sb[:], qps[:])

        # ---- qtld.T  [128 x 4 x 32] for the main-loop stationary ----
        qtldT = singles.tile([P, DC, BS], f32r)
        for dc in range(DC):
            tps = pre_ps.tile([P, BS], f32)
            nc.tensor.transpose(tps[:], qn_sb[:, dc * P:(dc + 1) * P], ident)
            nc.vector.tensor_copy(qtldT[:, dc, :], tps[:])

        pre_ps_cm.__exit__(None, None, None)

        # ---- main loop: logits[b, :] = qtldT[:, b] . Z_node[b].T ----
        zpool = ctx.enter_context(tc.tile_pool(name="z", bufs=zbufs))
        lps = ctx.enter_context(tc.tile_pool(name="lps", bufs=4, space="PSUM"))
        tpool = ctx.enter_context(tc.tile_pool(name="tanh", bufs=2))
        opool = ctx.enter_context(tc.tile_pool(name="orow", bufs=2))
        for b in range(BS):
            ztile = zpool.tile([P, DC, N], f32r)
            nc.sync.dma_start(ztile[:], zt[b].rearrange("dc p n -> p dc n"))
            ps = lps.tile([1, N], f32)
            for dc in range(DC):
                for nh in range(NH):
                    nc.tensor.matmul(
                        ps[:, nh * 512:(nh + 1) * 512],
                        qtldT[:, dc, b:b + 1],
                        ztile[:, dc, nh * 512:(nh + 1) * 512],
                        start=(dc == 0), stop=(dc == DC - 1))
            trow = tpool.tile([1, N], f32)
            nc.scalar.activation(trow[:], ps[:], Tanh, scale=1.0)
            orow = opool.tile([1, N], f32)
            nc.vector.tensor_scalar_mul(orow[:], trow[:], CLIP)
            nc.scalar.dma_start(out[b:b + 1, :], orow[:])

    nc.compile()
    return nc


def _get_nc(BS):
    if BS not in _CACHE:
        _CACHE[BS] = _build(BS)
    return _CACHE[BS]


def _make_in_maps(g_node, Z_veh, g_graph, Z_node, W_ctx, b_ctx, Wq, Wk, BS):
    ncores = g_node.shape[0] // BS
    w1t = np.ascontiguousarray(W_ctx.T)          # [3D, D], k-major
    w1t[D:2 * D, :] *= np.float32(1.0 / V)       # fold the Z_veh mean's 1/V
    # Weight-only fold: (ctx @ Wq.T) @ Wk == ctx @ (Wq.T @ Wk); also fold
    # the 1/sqrt(D) logit scale.  Computed in float64 for accuracy.
    wqk = ((Wq.T.astype(np.float64) @ Wk.astype(np.float64))
           / np.sqrt(D)).astype(np.float32)
    brep = np.broadcast_to(b_ctx, (32, D)).copy()
    eye = np.eye(32, dtype=np.float32)

    in_maps = []
    for c in range(ncores):
        s = slice(c * BS, (c + 1) * BS)
        ztc = np.ascontiguousarray(
            Z_node[s].transpose(0, 2, 1)).reshape(BS, DC, P, N)
        in_maps.append({
            "zt": ztc,
            "gn": np.ascontiguousarray(g_node[s]),
            "gg": np.ascontiguousarray(g_graph[s]),
            "zv": np.ascontiguousarray(Z_veh[s]),
            "w1t": w1t, "wqk": wqk, "brep": brep, "eye": eye,
        })
    return in_maps


def kernel(g_node, Z_veh, g_graph, Z_node, mask, W_ctx, b_ctx, Wq, Wk):
    from concourse.bass_utils import run_bass_kernel_spmd

    g_node = np.asarray(g_node, np.float32)
    Z_veh = np.asarray(Z_veh, np.float32)
    g_graph = np.asarray(g_graph, np.float32)
    Z_node = np.asarray(Z_node, np.float32)
    mask = np.asarray(mask, bool)
    W_ctx = np.asarray(W_ctx, np.float32)
    b_ctx = np.asarray(b_ctx, np.float32)
    Wq = np.asarray(Wq, np.float32)
    Wk = np.asarray(Wk, np.float32)

    BS = B // NCORES
    nc = _get_nc(BS)
    in_maps = _make_in_maps(g_node, Z_veh, g_graph, Z_node,
                            W_ctx, b_ctx, Wq, Wk, BS)
    res = run_bass_kernel_spmd(nc, in_maps, core_ids=list(range(NCORES)))
    logits = np.concatenate([r["out"] for r in res.results], axis=0)
    return np.where(mask, logits, np.float32(-np.inf)).astype(np.float32)


# revision 14
# speedup vs baseline: 2.6488x; 1.5521x over previous
"""Trainium2 Bass kernel for nn_NodeDecoder (sparse_attention).

Reference computation (B=256, V=16, N=1024, D=512):
    cat    = concat([g_node, Z_veh.mean(1), g_graph], -1)          # [B, 3D]
    ctx    = relu(cat @ W_ctx.T + b_ctx)                           # [B, D]
    Q      = ctx @ Wq.T                                            # [B, D]
    K      = Z_node @ Wk.T                                         # [B, N, D]
    logits = CLIP * tanh((Q . K) / sqrt(D)), masked to -inf        # [B, N]

Key algebraic transform: Q . (Z_node @ Wk.T) == (Q @ Wk) . Z_node, so the
B*N*D*D einsum collapses to small matmuls plus a B*N*D dot-product sweep.
The kernel is then HBM-bandwidth-bound on streaming Z_node exactly once.
Wq.T @ Wk (weights only) is folded into a single matrix host-side, as are
the 1/V of the Z_veh mean and the 1/sqrt(D) logit scale.

Distribution: data-parallel over batch B across 8 NeuronCores (32 b/core),
small weights replicated.  All activation FLOPs run on device; the host
only slices/relayouts inputs, preprocesses weights, and reassembles the
output (masked positions filled with -inf).

Per-core dataflow:
  - Z_veh sum over V: VectorE tree adds (contiguous slices)
  - cat.T via 12 PE identity-transposes -> catT [128 x 12 x 32]
  - ctx  = relu(catT.T @ W1T + b): 12 accumulating PE matmuls with the
    [128,32] catT chunk stationary and the [128,512] weight chunk moving
    (float32r -> full-rate streaming), bias+relu on VectorE
  - qtld = ctx @ (Wq.T @ Wk)/sqrt(D): transpose ctx, 4 more matmuls,
    transpose back into qtldT [128 x 4 x 32]
  - main loop over b: stream Z_node[b].T (host-relayouted [4,128,1024]) on
    the Sync HWDGE ring (kept free of any other DMA so the Z stream never
    stalls behind compute waits) and accumulate logits[b, :] in PSUM with
    the 128x1 q column stationary; tanh on ScalarE; rows bounce via DRAM
    scratch (Scalar ring) and get a final xCLIP on VectorE.  M=1 keeps
    every engine SBUF access at partition 0/32/64/96.
"""

import numpy as np
from contextlib import ExitStack

B, V, N, D = 256, 16, 1024, 512
NCORES = 8
CLIP = 10.0
P = 128
DC = D // P          # 4 chunks of 128 along D
KC = (3 * D) // P    # 12 chunks along 3D
NH = N // 512        # moving-operand halves of the node dim

_CACHE = {}


def _build(BS, c_pads, zbufs=6):
    """Build + compile the per-core Bass program for BS batches/core.

    c_pads[b] is the padded count of unmasked node columns for batch b
    (0 means the whole row is masked and is skipped entirely).
    """
    import concourse.bacc as bacc
    import concourse.tile as tile
    import concourse.mybir as mybir

    f32 = mybir.dt.float32
    f32r = mybir.dt.float32r
    Tanh = mybir.ActivationFunctionType.Tanh
    Add = mybir.AluOpType.add

    nc = bacc.Bacc("TRN2", target_bir_lowering=False, debug=False,
                   num_devices=NCORES)

    offs = np.concatenate([[0], np.cumsum([DC * P * c for c in c_pads])])
    zt = nc.dram_tensor("zt", [int(offs[-1])], f32r,
                        kind="ExternalInput").ap()
    gn = nc.dram_tensor("gn", [BS, D], f32, kind="ExternalInput").ap()
    gg = nc.dram_tensor("gg", [BS, D], f32, kind="ExternalInput").ap()
    zv = nc.dram_tensor("zv", [BS, V, D], f32, kind="ExternalInput").ap()
    w1t = nc.dram_tensor("w1t", [3 * D, D], f32r, kind="ExternalInput").ap()
    wqk = nc.dram_tensor("wqk", [D, D], f32r, kind="ExternalInput").ap()
    brep = nc.dram_tensor("brep", [32, D], f32, kind="ExternalInput").ap()
    eye = nc.dram_tensor("eye", [32, 32], f32, kind="ExternalInput").ap()
    out = nc.dram_tensor("out", [BS, N], f32, kind="ExternalOutput").ap()

    with tile.TileContext(nc) as tc, ExitStack() as ctx:
        singles = ctx.enter_context(tc.tile_pool(name="singles", bufs=1))
        pre = ctx.enter_context(tc.tile_pool(name="pre", bufs=1))
        pre_ps_cm = tc.tile_pool(name="pre_ps", bufs=2, space="PSUM")
        pre_ps = pre_ps_cm.__enter__()

        # ---- load replicated weights / per-core small inputs ----
        # All preamble DMAs sit at the HEAD of the Sync HWDGE ring, before
        # the Z_node stream: strict FIFO order means the (critical-path)
        # weights are delivered at full bandwidth instead of being starved
        # by a competing saturating queue.
        eye_sb = pre.tile([32, 32], f32)
        nc.sync.dma_start(eye_sb[:], eye[:])
        zv_sb = pre.tile([BS, V, D], f32)
        nc.sync.dma_start(zv_sb[:], zv[:])
        gn_sb = pre.tile([BS, D], f32)
        nc.sync.dma_start(gn_sb[:], gn[:])
        gg_sb = pre.tile([BS, D], f32)
        nc.sync.dma_start(gg_sb[:], gg[:])
        brep_sb = pre.tile([32, D], f32)
        nc.sync.dma_start(brep_sb[:], brep[:])
        w1t_sb = pre.tile([P, KC, D], f32r)
        nc.sync.dma_start(w1t_sb[:],
                          w1t.rearrange("(kc p) j -> p kc j", p=P))
        wqk_sb = pre.tile([P, DC, D], f32r)
        nc.sync.dma_start(wqk_sb[:],
                          wqk.rearrange("(kc p) j -> p kc j", p=P))

        ident = eye_sb[:BS, :BS]

        # ---- Z_veh sum over V: in-place VectorE tree adds ----
        h = V
        while h > 1:
            h //= 2
            nc.vector.tensor_tensor(zv_sb[:, 0:h, :], zv_sb[:, 0:h, :],
                                    zv_sb[:, h:2 * h, :], Add)
        zvsum = zv_sb[:, 0, :]

        # ---- cat.T  [3D partition-chunks x BS] via PE transposes ----
        catT = singles.tile([P, KC, BS], f32r)
        srcs = [gn_sb, zvsum, gg_sb]
        for g, src in enumerate(srcs):
            for dc in range(DC):
                tps = pre_ps.tile([P, BS], f32)
                nc.tensor.transpose(tps[:], src[:, dc * P:(dc + 1) * P], ident)
                nc.vector.tensor_copy(catT[:, g * DC + dc, :], tps[:])

        # ---- ctx = relu(cat @ W_ctx.T + b_ctx)   [32, 512] natural ----
        cps = pre_ps.tile([BS, D], f32)
        for kc in range(KC):
            nc.tensor.matmul(cps[:], catT[:, kc, :], w1t_sb[:, kc, :],
                             start=(kc == 0), stop=(kc == KC - 1))
        ctx_sb = pre.tile([BS, D], f32)
        nc.vector.tensor_tensor(ctx_sb[:], cps[:], brep_sb[:BS, :], Add)
        nc.vector.tensor_scalar_max(ctx_sb[:], ctx_sb[:], 0.0)

        # ---- ctx.T then qtld = ctx @ Wqk ----
        ctxT = pre.tile([P, DC, BS], f32r)
        for jc in range(DC):
            tps = pre_ps.tile([P, BS], f32)
            nc.tensor.transpose(tps[:], ctx_sb[:, jc * P:(jc + 1) * P], ident)
            nc.vector.tensor_copy(ctxT[:, jc, :], tps[:])
        qps = pre_ps.tile([BS, D], f32)
        for jc in range(DC):
            nc.tensor.matmul(qps[:], ctxT[:, jc, :], wqk_sb[:, jc, :],
                             start=(jc == 0), stop=(jc == DC - 1))
        qn_sb = pre.tile([BS, D], f32)
        nc.vector.tensor_copy(qn_sb[:], qps[:])

        # ---- qtld.T  [128 x 4 x 32] for the main-loop stationary ----
        qtldT = singles.tile([P, DC, BS], f32r)
        for dc in range(DC):
            tps = pre_ps.tile([P, BS], f32)
            nc.tensor.transpose(tps[:], qn_sb[:, dc * P:(dc + 1) * P], ident)
            nc.vector.tensor_copy(qtldT[:, dc, :], tps[:])

        pre_ps_cm.__exit__(None, None, None)

        # ---- main loop: logits[b, :] = qtldT[:, b] . Z_node[b].T ----
        zpool = ctx.enter_context(tc.tile_pool(name="z", bufs=zbufs))
        lps = ctx.enter_context(tc.tile_pool(name="lps", bufs=4, space="PSUM"))
        tpool = ctx.enter_context(tc.tile_pool(name="tanh", bufs=2))
        opool = ctx.enter_context(tc.tile_pool(name="orow", bufs=2))
        cmax = max(max(c_pads), 1)
        for b in range(BS):
            cp = c_pads[b]
            if cp == 0:
                continue
            ztile = zpool.tile([P, DC, cmax], f32r, tag="ztile")
            zsrc = zt[int(offs[b]):int(offs[b + 1])].rearrange(
                "(dc p n) -> p dc n", dc=DC, p=P)
            nc.sync.dma_start(ztile[:, :, :cp], zsrc)
            ps = lps.tile([1, cmax], f32, tag="ps")
            chunks = [(0, min(512, cp))]
            if cp > 512:
                chunks.append((512, cp - 512))
            for dc in range(DC):
                for o, ln in chunks:
                    nc.tensor.matmul(
                        ps[:, o:o + ln],
                        qtldT[:, dc, b:b + 1],
                        ztile[:, dc, o:o + ln],
                        start=(dc == 0), stop=(dc == DC - 1))
            trow = tpool.tile([1, cmax], f32, tag="trow")
            nc.scalar.activation(trow[:, :cp], ps[:, :cp], Tanh, scale=1.0)
            orow = opool.tile([1, cmax], f32, tag="orow")
            nc.vector.tensor_scalar_mul(orow[:, :cp], trow[:, :cp], CLIP)
            nc.scalar.dma_start(out[b:b + 1, :cp], orow[:, :cp])

    nc.compile()
    return nc


def _get_nc(BS, c_pads):
    key = (BS, tuple(map(int, c_pads)))
    if key not in _CACHE:
        _CACHE[key] = _build(BS, tuple(map(int, c_pads)))
    return _CACHE[key]


def _pack_z(Z_node_s, idxs, c_pads):
    """Pack unmasked node rows, transposed to [DC,P,c_pad] per batch."""
    BS = len(idxs)
    offs = np.concatenate([[0], np.cumsum([DC * P * c for c in c_pads])])
    flat = np.zeros(int(offs[-1]), np.float32)
    for b in range(BS):
        cp = c_pads[b]
        if cp == 0:
            continue
        blk = flat[int(offs[b]):int(offs[b + 1])].reshape(D, cp)
        blk[:, :len(idxs[b])] = Z_node_s[b, idxs[b], :].T
    return flat


def _make_in_maps(g_node, Z_veh, g_graph, Z_node, W_ctx, b_ctx, Wq, Wk, BS,
                  idxs_all, c_pads_all):
    ncores = g_node.shape[0] // BS
    w1t = np.ascontiguousarray(W_ctx.T)          # [3D, D], k-major
    w1t[D:2 * D, :] *= np.float32(1.0 / V)       # fold the Z_veh mean's 1/V
    # Weight-only fold: (ctx @ Wq.T) @ Wk == ctx @ (Wq.T @ Wk); also fold
    # the 1/sqrt(D) logit scale.  Computed in float64 for accuracy.
    wqk = ((Wq.T.astype(np.float64) @ Wk.astype(np.float64))
           / np.sqrt(D)).astype(np.float32)
    brep = np.broadcast_to(b_ctx, (32, D)).copy()
    eye = np.eye(32, dtype=np.float32)

    in_maps = []
    for c in range(ncores):
        s = slice(c * BS, (c + 1) * BS)
        ztc = _pack_z(Z_node[s], idxs_all[c], c_pads_all[c])
        in_maps.append({
            "zt": ztc,
            "gn": np.ascontiguousarray(g_node[s]),
            "gg": np.ascontiguousarray(g_graph[s]),
            "zv": np.ascontiguousarray(Z_veh[s]),
            "w1t": w1t, "wqk": wqk, "brep": brep, "eye": eye,
        })
    return in_maps


def kernel(g_node, Z_veh, g_graph, Z_node, mask, W_ctx, b_ctx, Wq, Wk):
    from concourse.bass_utils import run_bass_kernel_spmd

    g_node = np.asarray(g_node, np.float32)
    Z_veh = np.asarray(Z_veh, np.float32)
    g_graph = np.asarray(g_graph, np.float32)
    Z_node = np.asarray(Z_node, np.float32)
    mask = np.asarray(mask, bool)
    W_ctx = np.asarray(W_ctx, np.float32)
    b_ctx = np.asarray(b_ctx, np.float32)
    Wq = np.asarray(Wq, np.float32)
    Wk = np.asarray(Wk, np.float32)

    BS = B // NCORES
    idxs_all, c_pads_all = [], []
    for c in range(NCORES):
        idxs = [np.flatnonzero(mask[c * BS + i]) for i in range(BS)]
        idxs_all.append(idxs)
        c_pads_all.append([0 if len(ix) == 0 else -(-len(ix) // 16) * 16
                           for ix in idxs])
    # one program per core would need per-core shapes; SPMD shares one
    # program, so pad every core to the same per-slot counts
    c_pads = [max(c_pads_all[c][i] for c in range(NCORES))
              for i in range(BS)]
    c_pads_all = [list(c_pads) for _ in range(NCORES)]

    nc = _get_nc(BS, c_pads)
    in_maps = _make_in_maps(g_node, Z_veh, g_graph, Z_node,
                            W_ctx, b_ctx, Wq, Wk, BS,
                            idxs_all, c_pads_all)
    res = run_bass_kernel_spmd(nc, in_maps, core_ids=list(range(NCORES)))
    full = np.full((B, N), -np.inf, np.float32)
    for c in range(NCORES):
        dev = res.results[c]["out"]
        for i in range(BS):
            ix = idxs_all[c][i]
            if len(ix):
                full[c * BS + i, ix] = dev[i, :len(ix)]
    return full
